# revision 17
# baseline (speedup 1.0000x reference)
"""Trainium2 Bass kernel for bit-serial conv2d (nn_CustomConv2).

The reference's bit-serial inner loop collapses exactly to
    g(x, w) = trunc(x * w / 16)           (bits = 4)
so   out = relu(bias + sum_{i,j,c} trunc(x * w / 16)).

Since x in [0,16) and w in [-8,8), write |w| = a and decompose over a:
    trunc(x*w/16) = sum_{a=2..8} floor(x*a/16) * ([w==a] - [w==-a])
(a=1 contributes floor(x/16) = 0).  This linearizes the truncation into 7
"plane" activations A_a = floor(x*a/16) (small ints 0..7, exact in fp8 e4m3)
against {-1,0,1} masks derived from the weights, so the whole conv runs on
the PE array as fp8 matmuls.

Implementation highlights:
  - fp8 DoubleRow matmuls: two 128-row plane chunks are packed as the two
    k-tiles of one matmul (K=256) at 0.5 cyc/row -> 36 conv matmuls total,
    2 PSUM banks of 272 columns (flat windows, row-wrap lands in dead
    x=32,33 lanes the epilogue skips).
  - x is transposed/duplicated to [128, YXP] uint8 on the HOST.
  - planes are computed with a 2-op pipeline: ya = x*(a/16) + (192-0.46875)
    written to bf16 rounds to 192 + floor(x*a/16) (bf16 ulp at 192 is 1);
    pl = ya - 192 written to fp8.  Both ops hit the DVE 2x_2p mode; op_a for
    two chunks runs on ACT/Pool in parallel.
  - weights are host-packed contiguous -> big-descriptor DMAs, split into
    two SBUF tiles (exact deps) across Pool-SWDGE and ACT-HWDGE.
  - outputs leave per PSUM bank as [F, PIXB] via kv_writeback descriptors
    pre-generated on Pool (prepare_only) and fired by trigger_dma, so the
    tail pays only transfer + completion-sem, not HWDGE+DGE latency.
  - an early warmup matmul chain starts the PE pstate ramp clock so the
    conv matmuls run at the 2.4 GHz max clock.
"""

import numpy as np
import ml_dtypes

import concourse.bass as bass
import concourse.bacc as bacc
import concourse.mybir as mybir
from concourse.tile import TileContext
from concourse import bass_utils

F32 = mybir.dt.float32
FP8 = mybir.dt.float8e4
BF16 = mybir.dt.bfloat16
U8 = mybir.dt.uint8
I32 = mybir.dt.int32
FP8_NP = ml_dtypes.float8_e4m3

B, H, W, C, F = 4, 32, 32, 64, 128
KH = KW = 3
NCORES = 8
HL = H // 2          # output rows per core
YR = HL + 2          # input rows incl halo
XR = W + 2           # input cols incl pad
YX = YR * XR         # 612 spatial positions per core
YXP = 640            # padded
PIX = HL * W         # 512 output pixels per core
NPOS = KH * KW       # 9
NCHUNK = 4           # 128-row plane chunks; chunk t covers planes (2+2t, 3+2t)
CHUNK_A = [(2, 3), (4, 5), (6, 7), (8, 0)]
NSUP = 2             # DoubleRow super-chunks (2 chunks = 2 k-tiles each)
NBANK = 4            # PSUM banks; last kept small so its epilogue is short
HBS = [5, 5, 4, 2]   # output rows per bank
ROW0 = [0, 5, 10, 14]
NWS = [hb * XR for hb in HBS]   # flat window sizes (x=32,33 lanes dead)
WHALF = NPOS * 2 * F # 2304 weight columns per super-chunk

MAGIC = 192.0        # 1.5 * 2^7: bf16 round-to-int magic constant
OFF = MAGIC - 0.46875
N_WARMUP = 6         # PE pstate-ramp warmup matmuls


def _build_nc():
    nc = bacc.Bacc()
    xin = nc.dram_tensor("xin", [128, YXP], U8, kind="ExternalInput")
    wt0 = nc.dram_tensor("wt0", [128, WHALF], FP8, kind="ExternalInput")
    wt1 = nc.dram_tensor("wt1", [128, WHALF], FP8, kind="ExternalInput")
    bia = nc.dram_tensor("bia", [F, 1], F32, kind="ExternalInput")
    yout = nc.dram_tensor("yout", [128, PIX], BF16, kind="ExternalOutput")

    with TileContext(nc) as tc:
        with (
            tc.tile_pool(name="sb", bufs=1) as sb,
            tc.tile_pool(name="pacc", bufs=1, space="PSUM") as pacc,
            tc.tile_pool(name="pscr", bufs=1, space="PSUM") as pscr,
        ):
            # --- warmup: start the PE pstate ramp clock ASAP
            wz = sb.tile([128, 128], F32, tag="wz")
            nc.vector.memset(wz[:, :], 0.0)
            # dummy activation: hoists the ACT function-table load (1.3us)
            # to the very start, before x arrives
            dum = sb.tile([128, 1], F32, tag="dum")
            nc.vector.memset(dum[:, :], 0.0)
            nc.scalar.activation(out=dum[:, :], in_=dum[:, :],
                                 func=mybir.ActivationFunctionType.Relu,
                                 bias=0.0, scale=1.0)
            for _ in range(N_WARMUP):
                scr = pscr.tile([128, 128], F32, tag="scr")
                nc.tensor.matmul(scr[:, :], lhsT=wz[:, :], rhs=wz[:, :],
                                 start=True, stop=True)

            # --- input DMAs: x + bias on SP HWDGE; weights split into three
            # DMAs: s0 pos0-4 via Pool SWDGE (parallel descriptor gen, gates
            # the first matmuls), s0 pos5-8 via SP after x, s1 via ACT HWDGE
            xs = sb.tile([128, YXP], U8, tag="xs")
            nc.sync.dma_start(out=xs[:, :], in_=xin[:, :])
            W0A = 5 * 2 * F
            wsb0a = sb.tile([128, W0A], FP8, tag="wsb0a")
            wsb0b = sb.tile([128, WHALF - W0A], FP8, tag="wsb0b")
            wsb1 = sb.tile([128, WHALF], FP8, tag="wsb1")
            nc.gpsimd.dma_start(out=wsb0a[:, :], in_=wt0[:, 0:W0A])
            nc.sync.dma_start(out=wsb0b[:, :], in_=wt0[:, W0A:WHALF])
            nc.sync.dma_start(out=wsb1[:, :], in_=wt1[:, :])
            biast = sb.tile([128, 1], F32, tag="bias")
            nc.sync.dma_start(out=biast[:, :], in_=bia[:, :])

            # --- per-partition plane multipliers a/16 (a1 half in rows 64+)
            avs = []
            for t, (a0, a1) in enumerate(CHUNK_A):
                av = sb.tile([128, 1], F32, tag=f"av{t}", name=f"av{t}")
                nc.gpsimd.memset(av[0:64, :], a0 / 16.0)
                nc.gpsimd.memset(av[64:128, :], a1 / 16.0)
                avs.append(av)

            # --- planes
            ya = sb.tile([128, NCHUNK * YXP], BF16, tag="ya")
            pl = sb.tile([128, NCHUNK * YXP], FP8, tag="pl")

            HS = 384  # bank0 windows only read cols < 344

            def op_a(eng, t, lo, hi):
                if eng is nc.scalar:
                    eng.activation(
                        out=ya[:, t * YXP + lo:t * YXP + hi], in_=xs[:, lo:hi],
                        func=mybir.ActivationFunctionType.Copy,
                        bias=OFF, scale=avs[t][:, :])
                else:
                    eng.tensor_scalar(
                        out=ya[:, t * YXP + lo:t * YXP + hi], in0=xs[:, lo:hi],
                        scalar1=avs[t][:, :], scalar2=OFF,
                        op0=mybir.AluOpType.mult, op1=mybir.AluOpType.add)

            def op_b(eng, t, lo, hi):
                eng.tensor_scalar(
                    out=pl[:, t * YXP + lo:t * YXP + hi],
                    in0=ya[:, t * YXP + lo:t * YXP + hi],
                    scalar1=-MAGIC, scalar2=None, op0=mybir.AluOpType.add)

            op_a(nc.scalar, 1, 0, HS)       # ACT: a1h0 (feeds b1h0), h1s, a3
            op_a(nc.vector, 0, 0, HS)       # DVE: a0h0
            op_a(nc.scalar, 0, HS, YXP)
            op_a(nc.scalar, 1, HS, YXP)
            op_a(nc.gpsimd, 2, 0, YXP)      # Pool: a2, b2
            op_a(nc.scalar, 3, 0, HS)       # ACT: a3 in halves
            op_a(nc.scalar, 3, HS, YXP)
            op_b(nc.vector, 0, 0, HS)
            op_b(nc.vector, 1, 0, HS)
            op_b(nc.vector, 0, HS, YXP)
            op_b(nc.vector, 1, HS, YXP)
            op_b(nc.gpsimd, 2, 0, YXP)
            op_b(nc.vector, 3, 0, HS)       # DVE: b3 in halves
            op_b(nc.vector, 3, HS, YXP)

            # --- output path: PSUM accs, relu epilogue, prepped writebacks
            plv = pl[:, :].rearrange("r (t c) -> r t c", c=YXP)
            accs = [pacc.tile([128, NWS[bk]], F32, tag=f"acc{bk}",
                              name=f"acc{bk}") for bk in range(NBANK)]
            osb = sb.tile([128, PIX], BF16, tag="osb")

            def lhs(s, p):
                if s == 1:
                    wt, q = wsb1, p
                elif p < 5:
                    wt, q = wsb0a, p
                else:
                    wt, q = wsb0b, p - 5
                return wt[:, q * 2 * F:(q + 1) * 2 * F].rearrange(
                    "r (k f) -> r k f", f=F)

            def mm(s, bk):
                for p in range(NPOS):
                    i, j = divmod(p, KW)
                    base = (ROW0[bk] + i) * XR + j
                    nc.tensor.matmul(
                        accs[bk][:, :],
                        lhsT=lhs(s, p),
                        rhs=plv[:, 2 * s:2 * s + 2, base:base + NWS[bk]],
                        start=(s == 0 and p == 0),
                        stop=(s == NSUP - 1 and p == NPOS - 1),
                        perf_mode=mybir.MatmulPerfMode.DoubleRow,
                    )

            def epi(bk):
                osl = osb[:, ROW0[bk] * W:(ROW0[bk] + HBS[bk]) * W]
                if bk % 2 == 0:
                    # ACT relu (+bias)
                    nc.scalar.activation(
                        out=osl.rearrange("p (l x) -> p l x", x=W),
                        in_=accs[bk][:, :].rearrange(
                            "p (l x) -> p l x", x=XR)[:, :, 0:W],
                        func=mybir.ActivationFunctionType.Relu,
                        bias=biast[:, :], scale=1.0,
                    )
                else:
                    # DVE relu (+bias): alternate engines so staggered bank
                    # closes drain without queueing
                    nc.vector.tensor_scalar(
                        out=osl.rearrange("p (l x) -> p l x", x=W),
                        in0=accs[bk][:, :].rearrange(
                            "p (l x) -> p l x", x=XR)[:, :, 0:W],
                        scalar1=biast[:, :], scalar2=0.0,
                        op0=mybir.AluOpType.add, op1=mybir.AluOpType.max)

            for bk in range(NBANK):
                mm(0, bk)
            SPLIT = (ROW0[1] + HBS[1]) * W   # 320 px: banks 0-1 | 2-3
            for bk in range(NBANK):
                mm(1, bk)
                epi(bk)
                if bk == 1:
                    nc.gpsimd.dma_start(out=yout[:, 0:SPLIT],
                                        in_=osb[:, 0:SPLIT])
            nc.sync.dma_start(out=yout[:, SPLIT:PIX],
                              in_=osb[:, SPLIT:PIX])
    nc.finalize()
    return nc


_NC_CACHE = {}


def _get_nc():
    if "nc" not in _NC_CACHE:
        _NC_CACHE["nc"] = _build_nc()
    return _NC_CACHE["nc"]


def make_in_maps(inputs, kernel, bias):
    """Host-side sharding, x transpose/dup, weight-mask packing."""
    x = np.asarray(inputs, dtype=np.float32)
    k = np.asarray(kernel, dtype=np.float32)
    b = np.asarray(bias, dtype=np.float32)

    # masks per chunk: wh[t, pos, row=(half*64+c), f] = [w==a] - [w==-a]
    kf = k.reshape(NPOS, C, F)
    wh = np.zeros((NCHUNK, NPOS, 128, F), dtype=np.float32)
    for t, (a0, a1) in enumerate(CHUNK_A):
        for half, a in ((0, a0), (1, a1)):
            if a == 0:
                continue
            wh[t, :, half * 64:(half + 1) * 64, :] = (
                (kf == a).astype(np.float32) - (kf == -a).astype(np.float32)
            )
    # per super-chunk s: [row, pos, k, f] with chunk t = 2s + k
    wps = []
    for s in range(NSUP):
        wp = np.zeros((128, NPOS, 2, F), dtype=np.float32)
        for kk in range(2):
            wp[:, :, kk, :] = wh[2 * s + kk].transpose(1, 0, 2)
        wps.append(wp.reshape(128, WHALF).astype(FP8_NP))

    bia = np.ascontiguousarray(b.reshape(F, 1))

    xp = np.zeros((B, H + 2, W + 2, C), dtype=np.uint8)
    xp[:, 1:H + 1, 1:W + 1, :] = x.astype(np.uint8)
    in_maps = []
    for core in range(NCORES):
        bb, y0 = divmod(core, 2)
        sl = xp[bb, y0 * HL:y0 * HL + YR].reshape(YX, C).T  # [C, YX]
        xin = np.zeros((128, YXP), dtype=np.uint8)
        xin[0:64, 0:YX] = sl
        xin[64:128, 0:YX] = sl
        in_maps.append({"xin": xin, "wt0": wps[0], "wt1": wps[1], "bia": bia})
    return in_maps


def assemble(results):
    out = np.empty((B, H, W, F), dtype=np.float32)
    for core in range(NCORES):
        bb, y0 = divmod(core, 2)
        y = results[core]["yout"].astype(np.float32)
        out[bb, y0 * HL:(y0 + 1) * HL] = y.T.reshape(HL, W, F)
    return out


def run(inputs, kernel, bias, bits, trace=False, **spmd_kwargs):
    assert int(bits) == 4, f"kernel specialized for bits=4, got {bits}"
    nc = _get_nc()
    in_maps = make_in_maps(inputs, kernel, bias)
    res = bass_utils.run_bass_kernel_spmd(
        nc, in_maps, core_ids=list(range(NCORES)), trace=trace, **spmd_kwargs
    )
    return assemble(res.results), res


def kernel(**inputs):
    out, _ = run(inputs["inputs"], inputs["kernel"], inputs["bias"],
                 inputs["bits"], trace=False)
    return out


# revision 18
# speedup vs baseline: 1.0319x; 1.0319x over previous
"""Trainium2 Bass kernel for bit-serial conv2d (nn_CustomConv2).

The reference's bit-serial inner loop collapses exactly to
    g(x, w) = trunc(x * w / 16)           (bits = 4)
so   out = relu(bias + sum_{i,j,c} trunc(x * w / 16)).

Since x in [0,16) and w in [-8,8), write |w| = a and decompose over a:
    trunc(x*w/16) = sum_{a=2..8} floor(x*a/16) * ([w==a] - [w==-a])
(a=1 contributes floor(x/16) = 0).  This linearizes the truncation into 7
"plane" activations A_a = floor(x*a/16) (small ints 0..7, exact in fp8 e4m3)
against {-1,0,1} masks derived from the weights, so the whole conv runs on
the PE array as fp8 matmuls.

Implementation highlights:
  - fp8 DoubleRow matmuls: two 128-row plane chunks are packed as the two
    k-tiles of one matmul (K=256) at 0.5 cyc/row -> 36 conv matmuls total,
    2 PSUM banks of 272 columns (flat windows, row-wrap lands in dead
    x=32,33 lanes the epilogue skips).
  - x is transposed/duplicated to [128, YXP] uint8 on the HOST.
  - planes are computed with a 2-op pipeline: ya = x*(a/16) + (192-0.46875)
    written to bf16 rounds to 192 + floor(x*a/16) (bf16 ulp at 192 is 1);
    pl = ya - 192 written to fp8.  Both ops hit the DVE 2x_2p mode; op_a for
    two chunks runs on ACT/Pool in parallel.
  - weights are host-packed contiguous -> big-descriptor DMAs, split into
    two SBUF tiles (exact deps) across Pool-SWDGE and ACT-HWDGE.
  - outputs leave per PSUM bank as [F, PIXB] via kv_writeback descriptors
    pre-generated on Pool (prepare_only) and fired by trigger_dma, so the
    tail pays only transfer + completion-sem, not HWDGE+DGE latency.
  - an early warmup matmul chain starts the PE pstate ramp clock so the
    conv matmuls run at the 2.4 GHz max clock.
"""

import numpy as np
import ml_dtypes

import concourse.bass as bass
import concourse.bacc as bacc
import concourse.mybir as mybir
from concourse.tile import TileContext
from concourse import bass_utils

F32 = mybir.dt.float32
FP8 = mybir.dt.float8e4
BF16 = mybir.dt.bfloat16
U8 = mybir.dt.uint8
I32 = mybir.dt.int32
FP8_NP = ml_dtypes.float8_e4m3

B, H, W, C, F = 4, 32, 32, 64, 128
KH = KW = 3
NCORES = 8
HL = H // 2          # output rows per core
YR = HL + 2          # input rows incl halo
XR = W + 2           # input cols incl pad
YX = YR * XR         # 612 spatial positions per core
YXP = 640            # padded
PIX = HL * W         # 512 output pixels per core
NPOS = KH * KW       # 9
NCHUNK = 4           # 128-row plane chunks; chunk t covers planes (2+2t, 3+2t)
CHUNK_A = [(2, 3), (4, 5), (6, 7), (8, 0)]
NSUP = 2             # DoubleRow super-chunks (2 chunks = 2 k-tiles each)
NBANK = 4            # pixel-quarter PSUM banks
HBS = [4, 4, 4, 4]   # output rows per bank
ROW0 = [0, 4, 8, 12]
NWS = [hb * XR for hb in HBS]   # flat window sizes (x=32,33 lanes dead)
WHALF = NPOS * 2 * F # 2304 weight columns per super-chunk

MAGIC = 192.0        # 1.5 * 2^7: bf16 round-to-int magic constant
OFF = MAGIC - 0.46875
N_WARMUP = 6         # PE pstate-ramp warmup matmuls


def _build_nc():
    nc = bacc.Bacc()
    xin = nc.dram_tensor("xin", [128, YXP], U8, kind="ExternalInput")
    wt0 = nc.dram_tensor("wt0", [128, WHALF], FP8, kind="ExternalInput")
    wt1 = nc.dram_tensor("wt1", [128, WHALF], FP8, kind="ExternalInput")
    bia = nc.dram_tensor("bia", [F, 1], F32, kind="ExternalInput")
    yout = nc.dram_tensor("yout", [128, PIX], BF16, kind="ExternalOutput")

    with TileContext(nc) as tc:
        with (
            tc.tile_pool(name="sb", bufs=1) as sb,
            tc.tile_pool(name="pacc", bufs=1, space="PSUM") as pacc,
            tc.tile_pool(name="pscr", bufs=1, space="PSUM") as pscr,
        ):
            # --- warmup: start the PE pstate ramp clock ASAP
            wz = sb.tile([128, 128], F32, tag="wz")
            nc.vector.memset(wz[:, :], 0.0)
            # dummy activation: hoists the ACT function-table load (1.3us)
            # to the very start, before x arrives
            dum = sb.tile([128, 1], F32, tag="dum")
            nc.vector.memset(dum[:, :], 0.0)
            nc.scalar.activation(out=dum[:, :], in_=dum[:, :],
                                 func=mybir.ActivationFunctionType.Relu,
                                 bias=0.0, scale=1.0)
            for _ in range(N_WARMUP):
                scr = pscr.tile([128, 128], F32, tag="scr")
                nc.tensor.matmul(scr[:, :], lhsT=wz[:, :], rhs=wz[:, :],
                                 start=True, stop=True)

            # --- input DMAs: x + bias on SP HWDGE; weights split into three
            # DMAs: s0 pos0-4 via Pool SWDGE (parallel descriptor gen, gates
            # the first matmuls), s0 pos5-8 via SP after x, s1 via ACT HWDGE
            xs = sb.tile([128, YXP], U8, tag="xs")
            nc.sync.dma_start(out=xs[:, :], in_=xin[:, :])
            W0A = 5 * 2 * F
            wsb0a = sb.tile([128, W0A], FP8, tag="wsb0a")
            wsb0b = sb.tile([128, WHALF - W0A], FP8, tag="wsb0b")
            wsb1 = sb.tile([128, WHALF], FP8, tag="wsb1")
            nc.gpsimd.dma_start(out=wsb0a[:, :], in_=wt0[:, 0:W0A])
            nc.sync.dma_start(out=wsb0b[:, :], in_=wt0[:, W0A:WHALF])
            nc.sync.dma_start(out=wsb1[:, :], in_=wt1[:, :])
            biast = sb.tile([128, 1], F32, tag="bias")
            nc.sync.dma_start(out=biast[:, :], in_=bia[:, :])

            # --- per-partition plane multipliers a/16 (a1 half in rows 64+)
            avs = []
            for t, (a0, a1) in enumerate(CHUNK_A):
                av = sb.tile([128, 1], F32, tag=f"av{t}", name=f"av{t}")
                nc.gpsimd.memset(av[0:64, :], a0 / 16.0)
                nc.gpsimd.memset(av[64:128, :], a1 / 16.0)
                avs.append(av)

            # --- planes
            ya = sb.tile([128, NCHUNK * YXP], BF16, tag="ya")
            pl = sb.tile([128, NCHUNK * YXP], FP8, tag="pl")

            HS = 384  # bank0 windows only read cols < 344

            def op_a(eng, t, lo, hi):
                if eng is nc.scalar:
                    eng.activation(
                        out=ya[:, t * YXP + lo:t * YXP + hi], in_=xs[:, lo:hi],
                        func=mybir.ActivationFunctionType.Copy,
                        bias=OFF, scale=avs[t][:, :])
                else:
                    eng.tensor_scalar(
                        out=ya[:, t * YXP + lo:t * YXP + hi], in0=xs[:, lo:hi],
                        scalar1=avs[t][:, :], scalar2=OFF,
                        op0=mybir.AluOpType.mult, op1=mybir.AluOpType.add)

            def op_b(eng, t, lo, hi):
                eng.tensor_scalar(
                    out=pl[:, t * YXP + lo:t * YXP + hi],
                    in0=ya[:, t * YXP + lo:t * YXP + hi],
                    scalar1=-MAGIC, scalar2=None, op0=mybir.AluOpType.add)

            op_a(nc.scalar, 1, 0, HS)       # ACT: a1h0 (feeds b1h0), h1s, a3
            op_a(nc.vector, 0, 0, HS)       # DVE: a0h0
            op_a(nc.scalar, 0, HS, YXP)
            op_a(nc.scalar, 1, HS, YXP)
            op_a(nc.gpsimd, 2, 0, YXP)      # Pool: a2, b2
            op_a(nc.scalar, 3, 0, HS)       # ACT: a3 in halves
            op_a(nc.scalar, 3, HS, YXP)
            op_b(nc.vector, 0, 0, HS)
            op_b(nc.vector, 1, 0, HS)
            op_b(nc.vector, 0, HS, YXP)
            op_b(nc.vector, 1, HS, YXP)
            op_b(nc.gpsimd, 2, 0, YXP)
            op_b(nc.vector, 3, 0, HS)       # DVE: b3 in halves
            op_b(nc.vector, 3, HS, YXP)

            # --- output path: PSUM accs, relu epilogue, prepped writebacks
            plv = pl[:, :].rearrange("r (t c) -> r t c", c=YXP)
            accs = [pacc.tile([128, NWS[bk]], F32, tag=f"acc{bk}",
                              name=f"acc{bk}") for bk in range(NBANK)]
            osb = sb.tile([128, PIX], BF16, tag="osb")

            def lhs(s, p):
                if s == 1:
                    wt, q = wsb1, p
                elif p < 5:
                    wt, q = wsb0a, p
                else:
                    wt, q = wsb0b, p - 5
                return wt[:, q * 2 * F:(q + 1) * 2 * F].rearrange(
                    "r (k f) -> r k f", f=F)

            def mm(s, bk):
                for p in range(NPOS):
                    i, j = divmod(p, KW)
                    base = (ROW0[bk] + i) * XR + j
                    nc.tensor.matmul(
                        accs[bk][:, :],
                        lhsT=lhs(s, p),
                        rhs=plv[:, 2 * s:2 * s + 2, base:base + NWS[bk]],
                        start=(s == 0 and p == 0),
                        stop=(s == NSUP - 1 and p == NPOS - 1),
                        perf_mode=mybir.MatmulPerfMode.DoubleRow,
                    )

            def epi(bk):
                osl = osb[:, ROW0[bk] * W:(ROW0[bk] + HBS[bk]) * W]
                if bk % 2 == 0:
                    # ACT relu (+bias)
                    nc.scalar.activation(
                        out=osl.rearrange("p (l x) -> p l x", x=W),
                        in_=accs[bk][:, :].rearrange(
                            "p (l x) -> p l x", x=XR)[:, :, 0:W],
                        func=mybir.ActivationFunctionType.Relu,
                        bias=biast[:, :], scale=1.0,
                    )
                else:
                    # DVE relu (+bias): alternate engines so staggered bank
                    # closes drain without queueing
                    nc.vector.tensor_scalar(
                        out=osl.rearrange("p (l x) -> p l x", x=W),
                        in0=accs[bk][:, :].rearrange(
                            "p (l x) -> p l x", x=XR)[:, :, 0:W],
                        scalar1=biast[:, :], scalar2=0.0,
                        op0=mybir.AluOpType.add, op1=mybir.AluOpType.max)

            for bk in range(NBANK):
                mm(0, bk)
            SPLIT = (ROW0[1] + HBS[1]) * W   # 256 px: banks 0-1 | 2-3
            for bk in range(NBANK):
                mm(1, bk)
                epi(bk)
                if bk == 1:
                    nc.scalar.dma_start(out=yout[:, 0:SPLIT],
                                        in_=osb[:, 0:SPLIT])
            nc.sync.dma_start(out=yout[:, SPLIT:PIX],
                              in_=osb[:, SPLIT:PIX])
    nc.finalize()
    return nc


_NC_CACHE = {}


def _get_nc():
    if "nc" not in _NC_CACHE:
        _NC_CACHE["nc"] = _build_nc()
    return _NC_CACHE["nc"]


def make_in_maps(inputs, kernel, bias):
    """Host-side sharding, x transpose/dup, weight-mask packing."""
    x = np.asarray(inputs, dtype=np.float32)
    k = np.asarray(kernel, dtype=np.float32)
    b = np.asarray(bias, dtype=np.float32)

    # masks per chunk: wh[t, pos, row=(half*64+c), f] = [w==a] - [w==-a]
    kf = k.reshape(NPOS, C, F)
    wh = np.zeros((NCHUNK, NPOS, 128, F), dtype=np.float32)
    for t, (a0, a1) in enumerate(CHUNK_A):
        for half, a in ((0, a0), (1, a1)):
            if a == 0:
                continue
            wh[t, :, half * 64:(half + 1) * 64, :] = (
                (kf == a).astype(np.float32) - (kf == -a).astype(np.float32)
            )
    # per super-chunk s: [row, pos, k, f] with chunk t = 2s + k
    wps = []
    for s in range(NSUP):
        wp = np.zeros((128, NPOS, 2, F), dtype=np.float32)
        for kk in range(2):
            wp[:, :, kk, :] = wh[2 * s + kk].transpose(1, 0, 2)
        wps.append(wp.reshape(128, WHALF).astype(FP8_NP))

    bia = np.ascontiguousarray(b.reshape(F, 1))

    xp = np.zeros((B, H + 2, W + 2, C), dtype=np.uint8)
    xp[:, 1:H + 1, 1:W + 1, :] = x.astype(np.uint8)
    in_maps = []
    for core in range(NCORES):
        bb, y0 = divmod(core, 2)
        sl = xp[bb, y0 * HL:y0 * HL + YR].reshape(YX, C).T  # [C, YX]
        xin = np.zeros((128, YXP), dtype=np.uint8)
        xin[0:64, 0:YX] = sl
        xin[64:128, 0:YX] = sl
        in_maps.append({"xin": xin, "wt0": wps[0], "wt1": wps[1], "bia": bia})
    return in_maps


def assemble(results):
    out = np.empty((B, H, W, F), dtype=np.float32)
    for core in range(NCORES):
        bb, y0 = divmod(core, 2)
        y = results[core]["yout"].astype(np.float32)
        out[bb, y0 * HL:(y0 + 1) * HL] = y.T.reshape(HL, W, F)
    return out


def run(inputs, kernel, bias, bits, trace=False, **spmd_kwargs):
    assert int(bits) == 4, f"kernel specialized for bits=4, got {bits}"
    nc = _get_nc()
    in_maps = make_in_maps(inputs, kernel, bias)
    res = bass_utils.run_bass_kernel_spmd(
        nc, in_maps, core_ids=list(range(NCORES)), trace=trace, **spmd_kwargs
    )
    return assemble(res.results), res


def kernel(**inputs):
    out, _ = run(inputs["inputs"], inputs["kernel"], inputs["bias"],
                 inputs["bits"], trace=False)
    return out


# revision 19
# speedup vs baseline: 1.0612x; 1.0284x over previous
"""Trainium2 Bass kernel for bit-serial conv2d (nn_CustomConv2).

The reference's bit-serial inner loop collapses exactly to
    g(x, w) = trunc(x * w / 16)           (bits = 4)
so   out = relu(bias + sum_{i,j,c} trunc(x * w / 16)).

Since x in [0,16) and w in [-8,8), write |w| = a and decompose over a:
    trunc(x*w/16) = sum_{a=2..8} floor(x*a/16) * ([w==a] - [w==-a])
(a=1 contributes floor(x/16) = 0).  This linearizes the truncation into 7
"plane" activations A_a = floor(x*a/16) (small ints 0..7, exact in fp8 e4m3)
against {-1,0,1} masks derived from the weights, so the whole conv runs on
the PE array as fp8 matmuls.

Implementation highlights:
  - fp8 DoubleRow matmuls: two 128-row plane chunks are packed as the two
    k-tiles of one matmul (K=256) at 0.5 cyc/row -> 36 conv matmuls total,
    2 PSUM banks of 272 columns (flat windows, row-wrap lands in dead
    x=32,33 lanes the epilogue skips).
  - x is transposed/duplicated to [128, YXP] uint8 on the HOST.
  - planes are computed with a 2-op pipeline: ya = x*(a/16) + (192-0.46875)
    written to bf16 rounds to 192 + floor(x*a/16) (bf16 ulp at 192 is 1);
    pl = ya - 192 written to fp8.  Both ops hit the DVE 2x_2p mode; op_a for
    two chunks runs on ACT/Pool in parallel.
  - weights are host-packed contiguous -> big-descriptor DMAs, split into
    two SBUF tiles (exact deps) across Pool-SWDGE and ACT-HWDGE.
  - outputs leave per PSUM bank as [F, PIXB] via kv_writeback descriptors
    pre-generated on Pool (prepare_only) and fired by trigger_dma, so the
    tail pays only transfer + completion-sem, not HWDGE+DGE latency.
  - an early warmup matmul chain starts the PE pstate ramp clock so the
    conv matmuls run at the 2.4 GHz max clock.
"""

import numpy as np
import ml_dtypes

import concourse.bass as bass
import concourse.bacc as bacc
import concourse.mybir as mybir
from concourse.tile import TileContext
from concourse import bass_utils

F32 = mybir.dt.float32
FP8 = mybir.dt.float8e4
BF16 = mybir.dt.bfloat16
U8 = mybir.dt.uint8
I32 = mybir.dt.int32
FP8_NP = ml_dtypes.float8_e4m3

B, H, W, C, F = 4, 32, 32, 64, 128
KH = KW = 3
NCORES = 8
HL = H // 2          # output rows per core
YR = HL + 2          # input rows incl halo
XR = W + 2           # input cols incl pad
YX = YR * XR         # 612 spatial positions per core
YXP = 640            # padded
PIX = HL * W         # 512 output pixels per core
NPOS = KH * KW       # 9
NCHUNK = 4           # 128-row plane chunks; chunk t covers planes (2+2t, 3+2t)
CHUNK_A = [(2, 3), (4, 5), (6, 7), (8, 0)]
NSUP = 2             # DoubleRow super-chunks (2 chunks = 2 k-tiles each)
NBANK = 4            # pixel-quarter PSUM banks
HBS = [4, 4, 4, 4]   # output rows per bank
ROW0 = [0, 4, 8, 12]
NWS = [hb * XR for hb in HBS]   # flat window sizes (x=32,33 lanes dead)
WHALF = NPOS * 2 * F # 2304 weight columns per super-chunk

MAGIC = 192.0        # 1.5 * 2^7: bf16 round-to-int magic constant
OFF = MAGIC - 0.46875
N_WARMUP = 6         # PE pstate-ramp warmup matmuls


def _build_nc():
    nc = bacc.Bacc()
    xin = nc.dram_tensor("xin", [128, YXP], U8, kind="ExternalInput")
    wt0 = nc.dram_tensor("wt0", [128, WHALF], FP8, kind="ExternalInput")
    wt1 = nc.dram_tensor("wt1", [128, WHALF], FP8, kind="ExternalInput")
    bia = nc.dram_tensor("bia", [F, 1], F32, kind="ExternalInput")
    yout = nc.dram_tensor("yout", [128, PIX], BF16, kind="ExternalOutput")

    with TileContext(nc) as tc:
        with (
            tc.tile_pool(name="sb", bufs=1) as sb,
            tc.tile_pool(name="pacc", bufs=1, space="PSUM") as pacc,
            tc.tile_pool(name="pscr", bufs=1, space="PSUM") as pscr,
        ):
            # --- warmup: start the PE pstate ramp clock ASAP
            wz = sb.tile([128, 128], F32, tag="wz")
            nc.vector.memset(wz[:, :], 0.0)
            # dummy activation: hoists the ACT function-table load (1.3us)
            # to the very start, before x arrives
            dum = sb.tile([128, 1], F32, tag="dum")
            nc.vector.memset(dum[:, :], 0.0)
            nc.scalar.activation(out=dum[:, :], in_=dum[:, :],
                                 func=mybir.ActivationFunctionType.Relu,
                                 bias=0.0, scale=1.0)
            for _ in range(N_WARMUP):
                scr = pscr.tile([128, 128], F32, tag="scr")
                nc.tensor.matmul(scr[:, :], lhsT=wz[:, :], rhs=wz[:, :],
                                 start=True, stop=True)

            # --- input DMAs: x + bias on SP HWDGE; weights split into three
            # DMAs: s0 pos0-4 via Pool SWDGE (parallel descriptor gen, gates
            # the first matmuls), s0 pos5-8 via SP after x, s1 via ACT HWDGE
            xs = sb.tile([128, YXP], U8, tag="xs")
            nc.sync.dma_start(out=xs[:, :], in_=xin[:, :])
            W0A = 5 * 2 * F
            wsb0a = sb.tile([128, W0A], FP8, tag="wsb0a")
            wsb0b = sb.tile([128, WHALF - W0A], FP8, tag="wsb0b")
            wsb1 = sb.tile([128, WHALF], FP8, tag="wsb1")
            nc.gpsimd.dma_start(out=wsb0a[:, :], in_=wt0[:, 0:W0A])
            nc.sync.dma_start(out=wsb0b[:, :], in_=wt0[:, W0A:WHALF])
            nc.sync.dma_start(out=wsb1[:, :], in_=wt1[:, :])
            biast = sb.tile([128, 1], F32, tag="bias")
            nc.sync.dma_start(out=biast[:, :], in_=bia[:, :])

            # --- per-partition plane multipliers a/16 (a1 half in rows 64+)
            avs = []
            for t, (a0, a1) in enumerate(CHUNK_A):
                av = sb.tile([128, 1], F32, tag=f"av{t}", name=f"av{t}")
                nc.gpsimd.memset(av[0:64, :], a0 / 16.0)
                nc.gpsimd.memset(av[64:128, :], a1 / 16.0)
                avs.append(av)

            # --- planes
            ya = sb.tile([128, NCHUNK * YXP], BF16, tag="ya")
            pl = sb.tile([128, NCHUNK * YXP], FP8, tag="pl")

            HS = 384  # bank0 windows only read cols < 344

            def op_a(eng, t, lo, hi):
                if eng is nc.scalar:
                    eng.activation(
                        out=ya[:, t * YXP + lo:t * YXP + hi], in_=xs[:, lo:hi],
                        func=mybir.ActivationFunctionType.Copy,
                        bias=OFF, scale=avs[t][:, :])
                else:
                    eng.tensor_scalar(
                        out=ya[:, t * YXP + lo:t * YXP + hi], in0=xs[:, lo:hi],
                        scalar1=avs[t][:, :], scalar2=OFF,
                        op0=mybir.AluOpType.mult, op1=mybir.AluOpType.add)

            def op_b(eng, t, lo, hi):
                eng.tensor_scalar(
                    out=pl[:, t * YXP + lo:t * YXP + hi],
                    in0=ya[:, t * YXP + lo:t * YXP + hi],
                    scalar1=-MAGIC, scalar2=None, op0=mybir.AluOpType.add)

            op_a(nc.scalar, 1, 0, HS)       # ACT: a1 halves then a3 halves
            op_a(nc.vector, 0, 0, HS)       # DVE: a0 halves then all b01/b3
            op_a(nc.vector, 0, HS, YXP)
            op_a(nc.scalar, 1, HS, YXP)
            op_a(nc.gpsimd, 2, 0, HS)       # Pool: a2 then b2, in halves
            op_a(nc.scalar, 3, 0, HS)
            op_b(nc.vector, 0, 0, HS)
            op_b(nc.vector, 1, 0, HS)
            op_a(nc.gpsimd, 2, HS, YXP)
            op_a(nc.scalar, 3, HS, YXP)
            op_b(nc.vector, 0, HS, YXP)
            op_b(nc.vector, 1, HS, YXP)
            op_b(nc.gpsimd, 2, 0, HS)
            op_b(nc.vector, 3, 0, HS)
            op_b(nc.gpsimd, 2, HS, YXP)
            op_b(nc.vector, 3, HS, YXP)

            # --- output path: PSUM accs, relu epilogue, prepped writebacks
            plv = pl[:, :].rearrange("r (t c) -> r t c", c=YXP)
            accs = [pacc.tile([128, NWS[bk]], F32, tag=f"acc{bk}",
                              name=f"acc{bk}") for bk in range(NBANK)]
            osb = sb.tile([128, PIX], BF16, tag="osb")

            def lhs(s, p):
                if s == 1:
                    wt, q = wsb1, p
                elif p < 5:
                    wt, q = wsb0a, p
                else:
                    wt, q = wsb0b, p - 5
                return wt[:, q * 2 * F:(q + 1) * 2 * F].rearrange(
                    "r (k f) -> r k f", f=F)

            def mm(s, bk):
                for p in range(NPOS):
                    i, j = divmod(p, KW)
                    base = (ROW0[bk] + i) * XR + j
                    nc.tensor.matmul(
                        accs[bk][:, :],
                        lhsT=lhs(s, p),
                        rhs=plv[:, 2 * s:2 * s + 2, base:base + NWS[bk]],
                        start=(s == 0 and p == 0),
                        stop=(s == NSUP - 1 and p == NPOS - 1),
                        perf_mode=mybir.MatmulPerfMode.DoubleRow,
                    )

            def epi(bk):
                osl = osb[:, ROW0[bk] * W:(ROW0[bk] + HBS[bk]) * W]
                if bk % 2 == 0:
                    # ACT relu (+bias)
                    nc.scalar.activation(
                        out=osl.rearrange("p (l x) -> p l x", x=W),
                        in_=accs[bk][:, :].rearrange(
                            "p (l x) -> p l x", x=XR)[:, :, 0:W],
                        func=mybir.ActivationFunctionType.Relu,
                        bias=biast[:, :], scale=1.0,
                    )
                else:
                    # DVE relu (+bias): alternate engines so staggered bank
                    # closes drain without queueing
                    nc.vector.tensor_scalar(
                        out=osl.rearrange("p (l x) -> p l x", x=W),
                        in0=accs[bk][:, :].rearrange(
                            "p (l x) -> p l x", x=XR)[:, :, 0:W],
                        scalar1=biast[:, :], scalar2=0.0,
                        op0=mybir.AluOpType.add, op1=mybir.AluOpType.max)

            for bk in range(NBANK):
                mm(0, bk)
            SPLIT = HBS[0] * W   # 128 px: bank0 | banks 1-3
            for bk in range(NBANK):
                mm(1, bk)
                epi(bk)
                if bk == 0:
                    nc.sync.dma_start(out=yout[:, 0:SPLIT],
                                      in_=osb[:, 0:SPLIT])
            nc.sync.dma_start(out=yout[:, SPLIT:PIX],
                              in_=osb[:, SPLIT:PIX])
    nc.finalize()
    return nc


_NC_CACHE = {}


def _get_nc():
    if "nc" not in _NC_CACHE:
        _NC_CACHE["nc"] = _build_nc()
    return _NC_CACHE["nc"]


def make_in_maps(inputs, kernel, bias):
    """Host-side sharding, x transpose/dup, weight-mask packing."""
    x = np.asarray(inputs, dtype=np.float32)
    k = np.asarray(kernel, dtype=np.float32)
    b = np.asarray(bias, dtype=np.float32)

    # masks per chunk: wh[t, pos, row=(half*64+c), f] = [w==a] - [w==-a]
    kf = k.reshape(NPOS, C, F)
    wh = np.zeros((NCHUNK, NPOS, 128, F), dtype=np.float32)
    for t, (a0, a1) in enumerate(CHUNK_A):
        for half, a in ((0, a0), (1, a1)):
            if a == 0:
                continue
            wh[t, :, half * 64:(half + 1) * 64, :] = (
                (kf == a).astype(np.float32) - (kf == -a).astype(np.float32)
            )
    # per super-chunk s: [row, pos, k, f] with chunk t = 2s + k
    wps = []
    for s in range(NSUP):
        wp = np.zeros((128, NPOS, 2, F), dtype=np.float32)
        for kk in range(2):
            wp[:, :, kk, :] = wh[2 * s + kk].transpose(1, 0, 2)
        wps.append(wp.reshape(128, WHALF).astype(FP8_NP))

    bia = np.ascontiguousarray(b.reshape(F, 1))

    xp = np.zeros((B, H + 2, W + 2, C), dtype=np.uint8)
    xp[:, 1:H + 1, 1:W + 1, :] = x.astype(np.uint8)
    in_maps = []
    for core in range(NCORES):
        bb, y0 = divmod(core, 2)
        sl = xp[bb, y0 * HL:y0 * HL + YR].reshape(YX, C).T  # [C, YX]
        xin = np.zeros((128, YXP), dtype=np.uint8)
        xin[0:64, 0:YX] = sl
        xin[64:128, 0:YX] = sl
        in_maps.append({"xin": xin, "wt0": wps[0], "wt1": wps[1], "bia": bia})
    return in_maps


def assemble(results):
    out = np.empty((B, H, W, F), dtype=np.float32)
    for core in range(NCORES):
        bb, y0 = divmod(core, 2)
        y = results[core]["yout"].astype(np.float32)
        out[bb, y0 * HL:(y0 + 1) * HL] = y.T.reshape(HL, W, F)
    return out


def run(inputs, kernel, bias, bits, trace=False, **spmd_kwargs):
    assert int(bits) == 4, f"kernel specialized for bits=4, got {bits}"
    nc = _get_nc()
    in_maps = make_in_maps(inputs, kernel, bias)
    res = bass_utils.run_bass_kernel_spmd(
        nc, in_maps, core_ids=list(range(NCORES)), trace=trace, **spmd_kwargs
    )
    return assemble(res.results), res


def kernel(**inputs):
    out, _ = run(inputs["inputs"], inputs["kernel"], inputs["bias"],
                 inputs["bits"], trace=False)
    return out
